# revision 6
# baseline (speedup 1.0000x reference)
"""Trainium2 Bass kernel: log-odds transform + uniform-grid histogram binning.

Reference semantics (f32, bins = jnp.linspace(-8, 8, 4096), Xs in
[1e-3, 1-1e-3]):
    s   = log(Xs) - log(1 - Xs)
    idx = clip(searchsorted(bins, max(s, bins[0]), side='right') - 1, 0, 4095)
    out = bins[idx]              # straight-through forward value

Design (v5: single-Ln pipeline, three DMA paths, race-free DMA waits)
---------------------------------------------------------------------
ln(x) - ln(1-x) = -ln(1/x - 1), so per element:
    r = recip_approx_fast(x)          # DVE custom op (~26 ulp), in-place
    q = Ln(r - 1)                     # ACT; the -1 rides the free input bias
    k = u16((q - C1) * -invw)         # one affine + output-dtype cast
The f32->u16 output cast is round-to-nearest-even (HW-probed), so C1
bakes in a -0.5-bin offset to realize floor(). x in [1e-3, 1-1e-3]
bounds s to +-6.907, so k stays in [280, 3816]: no clamping needed.
One ACT pass instead of two (v1 was ACT-bound at 31.8us); the cast
runs on DVE for half the columns and as an ACT Copy-affine for tiles
{1,2,4,6} (both engines ~25.5us busy). The device emits u16 bin
indices; the host expands them through the caller-provided `bins`
table while unsharding (16KB table decode; all arithmetic on device).

DMA structure (HW-traced rationale):
  - The SDMA ramp is queue-depth-driven: a thin queue holds ~120GB/s
    for several us while a deep one reaches ~430GB/s by ~3us. So SYNC
    issues ALL in-DMAs back-to-back at t~7us with no gating (its HWDGE
    ring carries nothing else). Tile 0 goes as two 512KB halves so
    compute starts ~2us earlier; tiles 1-7 as 1MB transfers.
  - Mid-kernel out-DMAs issue from GPSIMD (SWDGE, its own queue -- the
    SDMA engines round-robin in/out packets, no FIFO coupling with the
    in stream). The final two outs (tile 7 halves) issue from SYNC,
    whose ring is empty by then: HWDGE receipt is what gates the end.
  - A DMA's completion sem gets +16 from 16 independent lanes and
    lanes of later DMAs can pass a straggler of an earlier one
    (observed as a stale partition-row), so every in-DMA has its OWN
    semaphore waited to exactly 16; the end-of-kernel wait is a full
    count over all out lanes, which cannot be masked.
  - ACT warm-ups (Ln + Copy) run before the compute loop so both
    ACT_TABLE_LOADs ride the first transfers' shadow.
  - DVE runs reciprocal jobs 3 ahead of the casts, casts first in each
    loop step so ready work never queues behind a data wait.
  - Tile 7 computes in 4x512-col chunks (short tail) but stores as
    2x256KB outs (tiny trailing DMAs have multi-us receipt trickle).
No SBUF slot reuse: 8 tiles x (8KB x + 8KB q + 4KB o) = 160KB/partition.

Accuracy: ~0.26% of elements shift by one bin (L2 rel err ~1.1e-4,
max abs err = one bin width) -- far inside the 2e-2 gate.
"""

from contextlib import ExitStack

import numpy as np

import concourse.bacc as bacc
import concourse.mybir as mybir
from concourse import bass_utils

N = 16_777_216
NCORES = 8
SHARD = N // NCORES
P = 128
FD = 2048
NT = 8
assert NT * P * FD == SHARD

NUM_BINS = 4096
INVW = float(np.float32(4095.0 / 16.0))
C1 = float(np.float32(8.0 - 8.0 / 4095.0))       # (q-C1)*-invw = (s+8)*invw - 0.5
BCOPY = float(np.float32(8.0 * INVW - 0.5))      # ACT-Copy: -invw*q + BCOPY
F32 = mybir.dt.float32
U16 = mybir.dt.uint16
Ln = mybir.ActivationFunctionType.Ln
Copy = mybir.ActivationFunctionType.Copy
Alu = mybir.AluOpType

A_TILES = frozenset({1, 2, 4, 6})   # affine+cast on ACT for these tiles

# compute jobs: (tile, col0, col1); tile 0 split in halves, tile 7 in quarters
JOBS = (
    [(0, 0, 1024), (0, 1024, 2048)]
    + [(t, 0, 2048) for t in range(1, 7)]
    + [(7, c, c + 512) for c in range(0, 2048, 512)]
)
NJ = len(JOBS)
# in-DMA index feeding job k (in-DMAs: 0 = t0[0:1024], 1 = t0[1024:2048],
# 2..8 = tiles 1..7)
IN_OF_JOB = [0, 1] + list(range(2, 9)) + [8, 8, 8]
N_DMA_IN = 9

# out-DMAs: (tile, col0, col1, jobs_needed); tile 7 stores in halves.
# The last two (tile 7) issue from sync; the rest from gpsimd.
OUTS = (
    [(0, 0, 1024, 1), (0, 1024, 2048, 2)]
    + [(t, 0, 2048, t + 2) for t in range(1, 7)]
    + [(7, 0, 1024, 10), (7, 1024, 2048, 12)]
)
N_OUT = len(OUTS)
N_OUT_GP = N_OUT - 2


def build_module():
    prod = ['a' if (j[0] in A_TILES) else 'd' for j in JOBS]
    cnt_d, cnt_a, cd, ca = [], [], 0, 0
    for p_ in prod:
        cd += p_ == 'd'
        ca += p_ == 'a'
        cnt_d.append(cd)
        cnt_a.append(ca)

    nc = bacc.Bacc("TRN2", target_bir_lowering=False, debug=False)
    x = nc.dram_tensor("x", [SHARD], F32, kind="ExternalInput")
    y = nc.dram_tensor("y", [SHARD], U16, kind="ExternalOutput")
    xv = x[:].rearrange("(n p m) -> n p m", p=P, m=FD)
    yv = y[:].rearrange("(n p m) -> n p m", p=P, m=FD)

    with ExitStack() as ctx:
        xb = ctx.enter_context(nc.sbuf_tensor("xb", [P, NT * FD], F32))
        qb = ctx.enter_context(nc.sbuf_tensor("qb", [P, NT * FD], F32))
        ob = ctx.enter_context(nc.sbuf_tensor("ob", [P, NT * FD], U16))
        bias = ctx.enter_context(nc.sbuf_tensor("bias", [P, 1], F32))
        warm = ctx.enter_context(nc.sbuf_tensor("warm", [P, 1], F32))
        in_sems = [
            ctx.enter_context(nc.semaphore(f"in{i}")) for i in range(N_DMA_IN)
        ]
        r_sem = ctx.enter_context(nc.semaphore("r_sem"))
        q_sem = ctx.enter_context(nc.semaphore("q_sem"))
        od_sem = ctx.enter_context(nc.semaphore("od_sem"))
        oa_sem = ctx.enter_context(nc.semaphore("oa_sem"))
        w_sem = ctx.enter_context(nc.semaphore("w_sem"))
        m_sem = ctx.enter_context(nc.semaphore("m_sem"))
        block = ctx.enter_context(nc.Block())

        def seg(buf, k):
            t, c0, c1 = JOBS[k]
            return buf[:, t * FD + c0:t * FD + c1]

        def out_gates(eng, need):
            nd = cnt_d[need - 1]
            na = cnt_a[need - 1]
            if nd:
                eng.wait_ge(od_sem, nd)
            if na:
                eng.wait_ge(oa_sem, na)

        @block.sync
        def _(sync):
            # the whole in stream, queued immediately: ramp + full prefetch
            sync.dma_start(xb[:, 0:1024], xv[0][:, 0:1024]).then_inc(in_sems[0], 16)
            sync.dma_start(xb[:, 1024:2048], xv[0][:, 1024:2048]).then_inc(
                in_sems[1], 16
            )
            for t in range(1, NT):
                sync.dma_start(
                    xb[:, t * FD:(t + 1) * FD], xv[t]
                ).then_inc(in_sems[t + 1], 16)
            # the final outs ride this (by now empty) ring: fast receipt
            for (t, c0, c1, need) in OUTS[-2:]:
                out_gates(sync, need)
                sync.dma_start(
                    yv[t][:, c0:c1], ob[:, t * FD + c0:t * FD + c1]
                ).then_inc(w_sem, 16)
            sync.wait_ge(w_sem, 16 * N_OUT)
            sync.sem_clear(w_sem)
            sync.sem_clear(od_sem)
            sync.sem_clear(oa_sem)

        @block.gpsimd
        def _(gp):
            for (t, c0, c1, need) in OUTS[:N_OUT_GP]:
                out_gates(gp, need)
                nc.gpsimd.dma_start(
                    yv[t][:, c0:c1], ob[:, t * FD + c0:t * FD + c1]
                ).then_inc(w_sem, 16)

        @block.scalar
        def _(scalar):
            scalar.wait_ge(m_sem, 1)
            # warm-ups pull both ACT_TABLE_LOADs into the transfers' shadow
            nc.scalar.activation(warm[:, :], bias[:, :], Ln, bias=bias[:, :])
            nc.scalar.activation(warm[:, :], bias[:, :], Copy, bias=0.0, scale=1.0)
            for k in range(NJ):
                scalar.wait_ge(r_sem, k + 1)
                nc.scalar.activation(
                    seg(qb, k), seg(xb, k), Ln, bias=bias[:, :]
                ).then_inc(q_sem, 1)
                if prod[k] == 'a':
                    nc.scalar.activation(
                        seg(ob, k), seg(qb, k), Copy, bias=BCOPY, scale=-INVW
                    ).then_inc(oa_sem, 1)
            scalar.sem_clear(r_sem)
            scalar.sem_clear(m_sem)

        @block.vector
        def _(vector):
            nc.vector.memset(bias[:, :], -1.0).then_inc(m_sem, 1)
            LOOK = 3

            def recip(j):
                vector.wait_ge(in_sems[IN_OF_JOB[j]], 16)
                nc.vector.reciprocal_approx_fast(
                    seg(xb, j), seg(xb, j)
                ).then_inc(r_sem, 1)

            for j in range(min(LOOK, NJ)):
                recip(j)
            for k in range(NJ):
                # cast first: its gate (Ln(k)) clears long before the
                # lookahead recip's data does
                if prod[k] == 'd':
                    vector.wait_ge(q_sem, k + 1)
                    nc.vector.tensor_scalar(
                        seg(ob, k), seg(qb, k), C1, -INVW,
                        Alu.subtract, Alu.mult,
                    ).then_inc(od_sem, 1)
                if k + LOOK < NJ:
                    recip(k + LOOK)
            for s in in_sems:
                vector.sem_clear(s)
            vector.sem_clear(q_sem)

    nc.compile()
    return nc


_module_cache = {}


def _get_module(**kwargs):
    key = repr(sorted(kwargs.items()))
    if key not in _module_cache:
        _module_cache[key] = build_module(**kwargs)
    return _module_cache[key]


def run(Xs, bins, trace=False, **build_kwargs):
    Xs = np.ascontiguousarray(np.asarray(Xs, dtype=np.float32))
    assert Xs.shape == (N,), Xs.shape
    bins_np = np.asarray(bins, dtype=np.float32)
    nc = _get_module(**build_kwargs)
    shards = Xs.reshape(NCORES, SHARD)
    in_maps = [{"x": shards[c]} for c in range(NCORES)]
    res = bass_utils.run_bass_kernel_spmd(
        nc, in_maps, core_ids=list(range(NCORES)), trace=trace
    )
    raw = np.concatenate([np.asarray(r["y"]) for r in res.results])
    out = np.take(bins_np, np.minimum(raw, NUM_BINS - 1).astype(np.int64))
    return out.astype(np.float32), res


def kernel(Xs, bins):
    out, _ = run(Xs, bins)
    return out


# revision 7
# speedup vs baseline: 1.0073x; 1.0073x over previous
"""Trainium2 Bass kernel: log-odds transform + uniform-grid histogram binning.

Reference semantics (f32, bins = jnp.linspace(-8, 8, 4096), Xs in
[1e-3, 1-1e-3]):
    s   = log(Xs) - log(1 - Xs)
    idx = clip(searchsorted(bins, max(s, bins[0]), side='right') - 1, 0, 4095)
    out = bins[idx]              # straight-through forward value

Design (v6: single-Ln pipeline, one backloaded DMA ring)
--------------------------------------------------------
ln(x) - ln(1-x) = -ln(1/x - 1), so per element:
    r = recip_approx_fast(x)          # DVE custom op (~26 ulp), in-place
    q = Ln(r - 1)                     # ACT; the -1 rides the free input bias
    k = u16((q - C1) * -invw)         # one affine + output-dtype cast
The f32->u16 output cast is round-to-nearest-even (HW-probed), so C1
bakes in a -0.5-bin offset to realize floor(). x in [1e-3, 1-1e-3]
bounds s to +-6.907, so k stays in [280, 3816]: no clamping needed.
One ACT pass instead of two (v1 was ACT-bound at 31.8us); the cast
runs on DVE for half the columns and as an ACT Copy-affine for tiles
{1,2,4,6} (both engines ~25.5us busy). The device emits u16 bin
indices; the host expands them through the caller-provided `bins`
table while unsharding (16KB table decode; all arithmetic on device).

DMA structure (HW-traced rationale): the end of the kernel is bounded
by T0 + (in+out bytes)/BW no matter how the streams interleave, so the
simplest optimal schedule is ONE HWDGE ring, strictly backloaded: SYNC
issues all 9 in-DMAs back-to-back at t~7us (deep queue => fast SDMA
ramp; ins drain at the full ~430GB/s with nothing competing), then the
out-DMAs queue FIFO behind them as their results complete, draining in
issue order after the last in-byte. ACT issues nothing -- warm-ups
(Ln + Copy ACT_TABLE_LOADs) ride the first transfers' shadow, then
pure compute. Tile 0 goes as two 512KB halves so compute starts ~1us
earlier; tile 7 computes in 4x512-col chunks (short tail) and stores
as 2x256KB outs (tiny trailing DMAs have multi-us receipt trickle).

DMA-wait correctness: a DMA's completion sem gets +16 from 16
independent SDMA lanes, and lanes of later DMAs can pass a straggler
lane of an earlier one (observed as a stale partition-row), so every
in-DMA has its OWN semaphore waited to exactly 16; the end-of-kernel
wait is a full count over all out lanes, which cannot be masked.
No SBUF slot reuse: 8 tiles x (8KB x + 8KB q + 4KB o) = 160KB/partition.

Accuracy: ~0.26% of elements shift by one bin (L2 rel err ~1.1e-4,
max abs err = one bin width) -- far inside the 2e-2 gate.
"""

from contextlib import ExitStack

import numpy as np

import concourse.bacc as bacc
import concourse.mybir as mybir
from concourse import bass_utils

N = 16_777_216
NCORES = 8
SHARD = N // NCORES
P = 128
FD = 2048
NT = 8
assert NT * P * FD == SHARD

NUM_BINS = 4096
INVW = float(np.float32(4095.0 / 16.0))
C1 = float(np.float32(8.0 - 8.0 / 4095.0))       # (q-C1)*-invw = (s+8)*invw - 0.5
BCOPY = float(np.float32(8.0 * INVW - 0.5))      # ACT-Copy: -invw*q + BCOPY
F32 = mybir.dt.float32
U16 = mybir.dt.uint16
Ln = mybir.ActivationFunctionType.Ln
Copy = mybir.ActivationFunctionType.Copy
Alu = mybir.AluOpType

A_TILES = frozenset({1, 2, 4, 6})   # affine+cast on ACT for these tiles

# compute jobs: (tile, col0, col1); tile 0 split in halves, tile 7 in quarters
JOBS = (
    [(0, 0, 1024), (0, 1024, 2048)]
    + [(t, 0, 2048) for t in range(1, 7)]
    + [(7, c, c + 512) for c in range(0, 2048, 512)]
)
NJ = len(JOBS)
# in-DMA index feeding job k (in-DMAs: 0 = t0[0:1024], 1 = t0[1024:2048],
# 2..8 = tiles 1..7)
IN_OF_JOB = [0, 1] + list(range(2, 9)) + [8, 8, 8]
N_DMA_IN = 9

# out-DMAs: (tile, col0, col1, jobs_needed); tile 7 stores in halves
OUTS = (
    [(0, 0, 1024, 1), (0, 1024, 2048, 2)]
    + [(t, 0, 2048, t + 2) for t in range(1, 7)]
    + [(7, 0, 1024, 10), (7, 1024, 2048, 12)]
)
N_OUT = len(OUTS)


def build_module():
    prod = ['a' if (j[0] in A_TILES) else 'd' for j in JOBS]
    cnt_d, cnt_a, cd, ca = [], [], 0, 0
    for p_ in prod:
        cd += p_ == 'd'
        ca += p_ == 'a'
        cnt_d.append(cd)
        cnt_a.append(ca)

    nc = bacc.Bacc("TRN2", target_bir_lowering=False, debug=False)
    x = nc.dram_tensor("x", [SHARD], F32, kind="ExternalInput")
    y = nc.dram_tensor("y", [SHARD], U16, kind="ExternalOutput")
    xv = x[:].rearrange("(n p m) -> n p m", p=P, m=FD)
    yv = y[:].rearrange("(n p m) -> n p m", p=P, m=FD)

    with ExitStack() as ctx:
        xb = ctx.enter_context(nc.sbuf_tensor("xb", [P, NT * FD], F32))
        qb = ctx.enter_context(nc.sbuf_tensor("qb", [P, NT * FD], F32))
        ob = ctx.enter_context(nc.sbuf_tensor("ob", [P, NT * FD], U16))
        bias = ctx.enter_context(nc.sbuf_tensor("bias", [P, 1], F32))
        warm = ctx.enter_context(nc.sbuf_tensor("warm", [P, 1], F32))
        in_sems = [
            ctx.enter_context(nc.semaphore(f"in{i}")) for i in range(N_DMA_IN)
        ]
        r_sem = ctx.enter_context(nc.semaphore("r_sem"))
        q_sem = ctx.enter_context(nc.semaphore("q_sem"))
        od_sem = ctx.enter_context(nc.semaphore("od_sem"))
        oa_sem = ctx.enter_context(nc.semaphore("oa_sem"))
        w_sem = ctx.enter_context(nc.semaphore("w_sem"))
        m_sem = ctx.enter_context(nc.semaphore("m_sem"))
        block = ctx.enter_context(nc.Block())

        def seg(buf, k):
            t, c0, c1 = JOBS[k]
            return buf[:, t * FD + c0:t * FD + c1]

        @block.sync
        def _(sync):
            # the whole in stream, queued immediately: fast ramp, full
            # prefetch, drains at line rate with nothing competing
            sync.dma_start(xb[:, 0:1024], xv[0][:, 0:1024]).then_inc(in_sems[0], 16)
            sync.dma_start(xb[:, 1024:2048], xv[0][:, 1024:2048]).then_inc(
                in_sems[1], 16
            )
            for t in range(1, NT):
                sync.dma_start(
                    xb[:, t * FD:(t + 1) * FD], xv[t]
                ).then_inc(in_sems[t + 1], 16)
            # outs queue FIFO behind the ins and drain after the last in-byte
            for (t, c0, c1, need) in OUTS:
                nd = cnt_d[need - 1]
                na = cnt_a[need - 1]
                if nd:
                    sync.wait_ge(od_sem, nd)
                if na:
                    sync.wait_ge(oa_sem, na)
                sync.dma_start(
                    yv[t][:, c0:c1], ob[:, t * FD + c0:t * FD + c1]
                ).then_inc(w_sem, 16)
            sync.wait_ge(w_sem, 16 * N_OUT)
            sync.sem_clear(w_sem)
            sync.sem_clear(od_sem)
            sync.sem_clear(oa_sem)

        @block.scalar
        def _(scalar):
            scalar.wait_ge(m_sem, 1)
            # warm-ups pull both ACT_TABLE_LOADs into the transfers' shadow
            nc.scalar.activation(warm[:, :], bias[:, :], Ln, bias=bias[:, :])
            nc.scalar.activation(warm[:, :], bias[:, :], Copy, bias=0.0, scale=1.0)
            for k in range(NJ):
                scalar.wait_ge(r_sem, k + 1)
                nc.scalar.activation(
                    seg(qb, k), seg(xb, k), Ln, bias=bias[:, :]
                ).then_inc(q_sem, 1)
                if prod[k] == 'a':
                    nc.scalar.activation(
                        seg(ob, k), seg(qb, k), Copy, bias=BCOPY, scale=-INVW
                    ).then_inc(oa_sem, 1)
            scalar.sem_clear(r_sem)
            scalar.sem_clear(m_sem)

        @block.vector
        def _(vector):
            nc.vector.memset(bias[:, :], -1.0).then_inc(m_sem, 1)
            LOOK = 3

            def recip(j):
                vector.wait_ge(in_sems[IN_OF_JOB[j]], 16)
                nc.vector.reciprocal_approx_fast(
                    seg(xb, j), seg(xb, j)
                ).then_inc(r_sem, 1)

            for j in range(min(LOOK, NJ)):
                recip(j)
            for k in range(NJ):
                # cast first: its gate (Ln(k)) clears long before the
                # lookahead recip's data does
                if prod[k] == 'd':
                    vector.wait_ge(q_sem, k + 1)
                    nc.vector.tensor_scalar(
                        seg(ob, k), seg(qb, k), C1, -INVW,
                        Alu.subtract, Alu.mult,
                    ).then_inc(od_sem, 1)
                if k + LOOK < NJ:
                    recip(k + LOOK)
            for s in in_sems:
                vector.sem_clear(s)
            vector.sem_clear(q_sem)

    nc.compile()
    return nc


_module_cache = {}


def _get_module(**kwargs):
    key = repr(sorted(kwargs.items()))
    if key not in _module_cache:
        _module_cache[key] = build_module(**kwargs)
    return _module_cache[key]


def run(Xs, bins, trace=False, **build_kwargs):
    Xs = np.ascontiguousarray(np.asarray(Xs, dtype=np.float32))
    assert Xs.shape == (N,), Xs.shape
    bins_np = np.asarray(bins, dtype=np.float32)
    nc = _get_module(**build_kwargs)
    shards = Xs.reshape(NCORES, SHARD)
    in_maps = [{"x": shards[c]} for c in range(NCORES)]
    res = bass_utils.run_bass_kernel_spmd(
        nc, in_maps, core_ids=list(range(NCORES)), trace=trace
    )
    raw = np.concatenate([np.asarray(r["y"]) for r in res.results])
    out = np.take(bins_np, np.minimum(raw, NUM_BINS - 1).astype(np.int64))
    return out.astype(np.float32), res


def kernel(Xs, bins):
    out, _ = run(Xs, bins)
    return out


# revision 8
# speedup vs baseline: 1.1097x; 1.1016x over previous
"""Trainium2 Bass kernel: log-odds transform + uniform-grid histogram binning.

Reference semantics (f32, bins = jnp.linspace(-8, 8, 4096), Xs in
[1e-3, 1-1e-3]):
    s   = log(Xs) - log(1 - Xs)
    idx = clip(searchsorted(bins, max(s, bins[0]), side='right') - 1, 0, 4095)
    out = bins[idx]              # straight-through forward value

Design (v7)
-----------
ln(x) - ln(1-x) = -ln(1/x - 1), so per element:
    r = recip_approx_fast(x)          # DVE custom op (~26 ulp), in-place
    q = Ln(r - 1)                     # ACT; the -1 rides the free input bias
    k = u16((q - C1) * -invw)         # one affine + output-dtype cast
The f32->u16 output cast is round-to-nearest-even (HW-probed), so C1
bakes in a -0.5-bin offset to realize floor(). x in [1e-3, 1-1e-3]
bounds s to +-6.907, so k stays in [280, 3816]: no clamping needed.
One ACT pass instead of two (v1 was ACT-bound at 31.8us); the cast
runs on DVE for half the columns and as an ACT Copy-affine for tiles
{1,2,4,6}, balancing DVE ~25us vs ACT ~25us. The device emits u16 bin
indices; the host expands them through the caller-provided `bins`
table while unsharding (16KB table decode; all arithmetic on device).

Schedule (HW-traced rationale):
  - ins on the ACT HWDGE ring (qActDynamicHW), outs on the SYNC ring
    (qSPDynamicHW): SDMA round-robins the two streams so outs drain
    throughout instead of bunching at the tail (out descriptors are
    4KB/partition and drain at ~half rate on their own).
  - ACT issues in-DMAs for tile 0's halves + tile 1 first, then the Ln
    + Copy warm-ups (both ACT_TABLE_LOADs ride those transfers'
    shadow), then the remaining ins -- the deep queue keeps the SDMA
    ramp fast and the issue stream never blocks compute: by the time
    any tile is needed the queue is hot.
  - tile 0 AND tile 7 go as 512KB half-tile transfers: tile 0's halves
    start compute ~1us earlier; tile 7's halves shorten the post-
    last-in dependency chain (recip->Ln->cast at 1024 cols each).
  - a DMA's completion semaphore gets +16 from 16 independent SDMA
    lanes, and lanes of later DMAs can pass a straggler lane of an
    earlier one (observed as a stale partition-row): every in-DMA has
    its OWN semaphore waited to exactly 16; the end-of-kernel wait is
    a full count over all out lanes, which cannot be masked.
  - DVE runs reciprocals 3 jobs ahead of the casts, casts first in
    each loop step, so ready work never queues behind a data wait.
No SBUF slot reuse: 8 tiles x (8KB x + 8KB q + 4KB o) = 160KB/partition.

Accuracy: ~0.26% of elements shift by one bin (L2 rel err ~1.1e-4,
max abs err = one bin width) -- far inside the 2e-2 gate.
"""

from contextlib import ExitStack

import numpy as np

import concourse.bacc as bacc
import concourse.mybir as mybir
from concourse import bass_utils

N = 16_777_216
NCORES = 8
SHARD = N // NCORES
P = 128
FD = 2048
NT = 8
assert NT * P * FD == SHARD

NUM_BINS = 4096
INVW = float(np.float32(4095.0 / 16.0))
C1 = float(np.float32(8.0 - 8.0 / 4095.0))       # (q-C1)*-invw = (s+8)*invw - 0.5
BCOPY = float(np.float32(8.0 * INVW - 0.5))      # ACT-Copy: -invw*q + BCOPY
F32 = mybir.dt.float32
U16 = mybir.dt.uint16
Ln = mybir.ActivationFunctionType.Ln
Copy = mybir.ActivationFunctionType.Copy
Alu = mybir.AluOpType

A_TILES = frozenset({1, 2, 4, 6})   # affine+cast on ACT for these tiles

# compute jobs: (tile, col0, col1); tiles 0 and 7 split in halves
JOBS = (
    [(0, 0, 1024), (0, 1024, 2048)]
    + [(t, 0, 2048) for t in range(1, 7)]
    + [(7, 0, 1024), (7, 1024, 2048)]
)
NJ = len(JOBS)
# in-DMAs: 0 = t0[0:1024], 1 = t0[1024:2048], 2..7 = tiles 1..6,
#          8 = t7[0:1024], 9 = t7[1024:2048]   (one per job)
IN_OF_JOB = list(range(10))
N_DMA_IN = 10

# out-DMAs: one per job (tile, col0, col1, jobs_needed)
OUTS = [(t, c0, c1, k + 1) for k, (t, c0, c1) in enumerate(JOBS)]
N_OUT = len(OUTS)


def build_module():
    prod = ['a' if (j[0] in A_TILES) else 'd' for j in JOBS]
    cnt_d, cnt_a, cd, ca = [], [], 0, 0
    for p_ in prod:
        cd += p_ == 'd'
        ca += p_ == 'a'
        cnt_d.append(cd)
        cnt_a.append(ca)

    nc = bacc.Bacc("TRN2", target_bir_lowering=False, debug=False)
    x = nc.dram_tensor("x", [SHARD], F32, kind="ExternalInput")
    y = nc.dram_tensor("y", [SHARD], U16, kind="ExternalOutput")
    xv = x[:].rearrange("(n p m) -> n p m", p=P, m=FD)
    yv = y[:].rearrange("(n p m) -> n p m", p=P, m=FD)

    with ExitStack() as ctx:
        xb = ctx.enter_context(nc.sbuf_tensor("xb", [P, NT * FD], F32))
        qb = ctx.enter_context(nc.sbuf_tensor("qb", [P, NT * FD], F32))
        ob = ctx.enter_context(nc.sbuf_tensor("ob", [P, NT * FD], U16))
        bias = ctx.enter_context(nc.sbuf_tensor("bias", [P, 1], F32))
        warm = ctx.enter_context(nc.sbuf_tensor("warm", [P, 1], F32))
        in_sems = [
            ctx.enter_context(nc.semaphore(f"in{i}")) for i in range(N_DMA_IN)
        ]
        r_sem = ctx.enter_context(nc.semaphore("r_sem"))
        q_sem = ctx.enter_context(nc.semaphore("q_sem"))
        od_sem = ctx.enter_context(nc.semaphore("od_sem"))
        oa_sem = ctx.enter_context(nc.semaphore("oa_sem"))
        w_sem = ctx.enter_context(nc.semaphore("w_sem"))
        m_sem = ctx.enter_context(nc.semaphore("m_sem"))
        block = ctx.enter_context(nc.Block())

        def seg(buf, k):
            t, c0, c1 = JOBS[k]
            return buf[:, t * FD + c0:t * FD + c1]

        def dma_in(k):
            t, c0, c1 = JOBS[k]
            return nc.scalar.dma_start(
                xb[:, t * FD + c0:t * FD + c1], xv[t][:, c0:c1]
            ).then_inc(in_sems[k], 16)

        @block.sync
        def _(sync):
            for k, (t, c0, c1, need) in enumerate(OUTS):
                nd = cnt_d[need - 1]
                na = cnt_a[need - 1]
                if nd:
                    sync.wait_ge(od_sem, nd)
                if na:
                    sync.wait_ge(oa_sem, na)
                sync.dma_start(
                    yv[t][:, c0:c1], ob[:, t * FD + c0:t * FD + c1]
                ).then_inc(w_sem, 16)
            sync.wait_ge(w_sem, 16 * N_OUT)
            sync.sem_clear(w_sem)
            sync.sem_clear(od_sem)
            sync.sem_clear(oa_sem)

        @block.scalar
        def _(scalar):
            scalar.wait_ge(m_sem, 1)
            for k in (0, 1, 2):
                dma_in(k)
            # warm-ups pull both ACT_TABLE_LOADs into the transfers' shadow
            nc.scalar.activation(warm[:, :], bias[:, :], Ln, bias=bias[:, :])
            nc.scalar.activation(warm[:, :], bias[:, :], Copy, bias=0.0, scale=1.0)
            for k in range(3, NJ):
                dma_in(k)
            for k in range(NJ):
                scalar.wait_ge(r_sem, k + 1)
                nc.scalar.activation(
                    seg(qb, k), seg(xb, k), Ln, bias=bias[:, :]
                ).then_inc(q_sem, 1)
                if prod[k] == 'a':
                    nc.scalar.activation(
                        seg(ob, k), seg(qb, k), Copy, bias=BCOPY, scale=-INVW
                    ).then_inc(oa_sem, 1)
            scalar.sem_clear(r_sem)
            scalar.sem_clear(m_sem)

        @block.vector
        def _(vector):
            nc.vector.memset(bias[:, :], -1.0).then_inc(m_sem, 1)
            LOOK = 3

            def recip(j):
                vector.wait_ge(in_sems[IN_OF_JOB[j]], 16)
                nc.vector.reciprocal_approx_fast(
                    seg(xb, j), seg(xb, j)
                ).then_inc(r_sem, 1)

            for j in range(min(LOOK, NJ)):
                recip(j)
            for k in range(NJ):
                # cast first: its gate (Ln(k)) clears long before the
                # lookahead recip's data does
                if prod[k] == 'd':
                    vector.wait_ge(q_sem, k + 1)
                    nc.vector.tensor_scalar(
                        seg(ob, k), seg(qb, k), C1, -INVW,
                        Alu.subtract, Alu.mult,
                    ).then_inc(od_sem, 1)
                if k + LOOK < NJ:
                    recip(k + LOOK)
            for s in in_sems:
                vector.sem_clear(s)
            vector.sem_clear(q_sem)

    nc.compile()
    return nc


_module_cache = {}


def _get_module(**kwargs):
    key = repr(sorted(kwargs.items()))
    if key not in _module_cache:
        _module_cache[key] = build_module(**kwargs)
    return _module_cache[key]


def run(Xs, bins, trace=False, **build_kwargs):
    Xs = np.ascontiguousarray(np.asarray(Xs, dtype=np.float32))
    assert Xs.shape == (N,), Xs.shape
    bins_np = np.asarray(bins, dtype=np.float32)
    nc = _get_module(**build_kwargs)
    shards = Xs.reshape(NCORES, SHARD)
    in_maps = [{"x": shards[c]} for c in range(NCORES)]
    res = bass_utils.run_bass_kernel_spmd(
        nc, in_maps, core_ids=list(range(NCORES)), trace=trace
    )
    raw = np.concatenate([np.asarray(r["y"]) for r in res.results])
    out = np.take(bins_np, np.minimum(raw, NUM_BINS - 1).astype(np.int64))
    return out.astype(np.float32), res


def kernel(Xs, bins):
    out, _ = run(Xs, bins)
    return out
